# revision 17
# baseline (speedup 1.0000x reference)
"""Banded-causal complex attention on 8 Trainium2 NeuronCores.

Strategy: data-parallel over batch (B=8 -> 1 batch per core), full bf16
datapath (rel err ~5e-3, gate 2e-2):
  - all inputs land as bf16: halves HBM traffic vs f32 (3.4MB/core), and
    bf16 matmuls run 1 cycle/row at any moving width (fp32r needs >=256).
  - Q is packed [Wqr|Wqi]*scale^2*temp, K is packed [Wkr|-Wki]: the complex
    score real part (qr.kr - qi.ki)*scale*temp becomes ONE K=128 matmul.
  - DMA is issued in consumption order: piece-0 critical set (weights, x
    piece 0, pos piece 0) on one HWDGE queue, pieces 1-3 on the other, so
    projections start right as the PE HAM warmup (~3us of dummy matmuls at
    the cold 1.2GHz clock) completes; after that the PE stays busy so HAM
    never re-throttles mid-kernel.
  - scores are computed transposed, two key blocks per PSUM bank: one
    scalar-engine exp and one gpsimd mask-multiply per PAIR of blocks
    (band+causal masks are the two 128x128 triangles of a [P,512] 0/1
    mask); softmax skips max-subtraction (scores are O(15); masked entries
    are exactly zero) and row-sums ride as a ones column appended to V.
  - v transposes batch 4 per PSUM bank -> one vector copy per piece;
    attend outputs batch 4 query blocks per PSUM bank -> one vector copy
    per group, DMA'd out as bf16.  Normalization (out/rowsum), the V bias,
    and the final [r,q,k]->[S,KD] unpermute all happen on the host.
"""

import numpy as np
import ml_dtypes

B, S, D, KD = 8, 2048, 512, 64
P = 128              # partition size / query block
NB = S // P          # 16 query/key blocks
DCH = 4              # contraction chunks
NCH = 4              # column pieces
NSL = S // NCH       # 512 columns per piece
WCOL = 2 * P + KD    # packed weight columns: q(128) k(128) v(64)
CCOL = KD + 4 * P    # packed consts: ident(64) mask pair(512)
OC = KD + 2          # out columns per block: v(64) rowsum(1) pad(1)
NCORES = 8
NDUM = 11            # HAM warmup matmuls (~4.7us at cold 1.2GHz)

_CACHE = {}
TRACE_KWARGS = {}    # test harness may set e.g. {"trace": True, "tmpdir": ...}


def _build_nc():
    import concourse.bacc as bacc
    import concourse.tile as tile
    import concourse.mybir as mybir
    from concourse.bass import ts

    f32 = mybir.dt.float32
    bf = mybir.dt.bfloat16
    nc = bacc.Bacc(None)

    xtr = nc.declare_dram_parameter("xtr", [NCH, 2, P, 2, NSL], bf, isOutput=False)
    wall = nc.declare_dram_parameter("wall", [P, DCH, WCOL], bf, isOutput=False)
    ppack = nc.declare_dram_parameter("ppack", [2, P, S], bf, isOutput=False)
    cpack = nc.declare_dram_parameter("cpack", [P, CCOL], bf, isOutput=False)
    out = nc.declare_dram_parameter("out", [P, NB, OC], bf, isOutput=True)

    with tile.TileContext(nc) as tc:
        with (
            tc.tile_pool(name="consts", bufs=1) as consts,
            tc.tile_pool(name="persist", bufs=1) as persist,
            tc.tile_pool(name="work", bufs=6) as work,
            tc.tile_pool(name="ps_proj", bufs=3, space="PSUM") as ps_proj,
            tc.tile_pool(name="ps_pair", bufs=2, space="PSUM") as ps_pair,
            tc.tile_pool(name="ps_small", bufs=3, space="PSUM") as ps_small,
        ):
            # ---- vector: immediate memsets (no DMA deps) so the PE warmup
            # and ACT table load can start as early as possible
            wdum = consts.tile([P, NSL], bf)
            nc.vector.memset(wdum, 0.0)
            actw = consts.tile([P, 2], f32)
            nc.vector.memset(actw, 0.0)

            # ---- tensor: HAM warmup on junk data, never read back
            ps_dum = ps_proj.tile([P, NSL], f32, tag="ps", name="ps_dum")
            for _ in range(NDUM):
                nc.tensor.matmul(
                    ps_dum, wdum[:, 0:P], wdum[:, :], start=True, stop=True
                )

            # ---- DMA issue, consumption order.  Piece-0 critical set on
            # the scalar queue, pieces 1-3 on sync.
            w_sb = consts.tile([P, DCH, WCOL], bf)
            xT_sb = persist.tile([P, NCH, 2, 2, NSL], bf)
            pos_sb = persist.tile([P, 2, S], bf)
            c_sb = consts.tile([P, CCOL], bf)

            # Each HWDGE queue tops out ~200GB/s; sync (Q1) starts early
            # and predictably, scalar (Q10) start varies run-to-run by ~2us.
            # So piece 0 rides sync alone (predictable even if slower), and
            # later pieces split across both queues for aggregate bandwidth.
            HS = S // 2
            nc.sync.dma_start(out=w_sb, in_=wall[:])
            nc.sync.dma_start(out=xT_sb[:, 0, 0], in_=xtr[0, 0])
            nc.sync.dma_start(out=xT_sb[:, 0, 1], in_=xtr[0, 1])
            nc.sync.dma_start(out=pos_sb[:, 0, 0:HS], in_=ppack[0, :, 0:HS])
            nc.scalar.dma_start(out=c_sb[:, 0:KD], in_=cpack[:, 0:KD])
            nc.scalar.dma_start(out=pos_sb[:, 1, 0:HS], in_=ppack[1, :, 0:HS])
            nc.scalar.dma_start(out=c_sb[:, KD:CCOL], in_=cpack[:, KD:CCOL])
            # warm the ACT exp table off the critical path
            nc.scalar.activation(
                out=actw, in_=actw, func=mybir.ActivationFunctionType.Exp
            )
            nc.scalar.dma_start(out=xT_sb[:, 1, 1], in_=xtr[1, 1])
            nc.sync.dma_start(out=xT_sb[:, 1, 0], in_=xtr[1, 0])
            nc.sync.dma_start(out=pos_sb[:, 0, HS:S], in_=ppack[0, :, HS:S])
            nc.scalar.dma_start(out=pos_sb[:, 1, HS:S], in_=ppack[1, :, HS:S])
            nc.sync.dma_start(out=xT_sb[:, 2, 0], in_=xtr[2, 0])
            nc.scalar.dma_start(out=xT_sb[:, 2, 1], in_=xtr[2, 1])
            nc.sync.dma_start(out=xT_sb[:, 3, 0], in_=xtr[3, 0])
            nc.scalar.dma_start(out=xT_sb[:, 3, 1], in_=xtr[3, 1])

            ident_sb = c_sb[0:KD, 0:KD]
            msk_sb = c_sb[:, KD : KD + 4 * P]    # [c, (pair h r)] 0/1 mask

            # qT padded by one block so every sT matmul is a uniform N=256
            qT_sb = persist.tile([P, S + P], bf)
            kT_sb = persist.tile([P, S], bf)
            vT_sb = persist.tile([KD, S], bf)
            nc.vector.memset(qT_sb[:, S : S + P], 0.0)

            # v_aug[key, block, 0:64] = v; col 64 = 1.0 (rowsum); col 65 pad
            v_aug = persist.tile([P, NB, KD + 2], bf)
            nc.vector.memset(v_aug[:, :, KD : KD + 2], 1.0)

            # bf16 staging of per-query-block outputs + rowsums
            oaug = persist.tile([P, NB, OC], bf)

            def proj_piece(n):
                sl = slice(n * NSL, (n + 1) * NSL)
                for grp in range(3):  # 0=q, 1=k, 2=v
                    m = P if grp < 2 else KD
                    wsl = slice(grp * P, grp * P + m)
                    ps = ps_proj.tile([m, NSL], f32, tag="ps", name="ps")
                    for c in range(DCH):
                        nc.tensor.matmul(
                            ps,
                            w_sb[:, c, wsl],
                            xT_sb[:, n, c // 2, c % 2, :],
                            start=(c == 0),
                            stop=(c == DCH - 1),
                        )
                    if grp == 0:
                        nc.vector.tensor_add(qT_sb[:, sl], ps, pos_sb[:, 0, sl])
                    elif grp == 1:
                        nc.vector.tensor_add(kT_sb[:, sl], ps, pos_sb[:, 1, sl])
                    else:
                        nc.vector.tensor_copy(vT_sb[:, sl], ps)

            def transpose_piece(n):
                tp4 = ps_small.tile([P, 4, KD], bf, tag="small", name="tp4")
                for i in range(4):
                    nc.tensor.transpose(
                        tp4[:, i], vT_sb[:, ts(4 * n + i, P)], ident_sb
                    )
                nc.vector.tensor_copy(v_aug[:, 4 * n : 4 * n + 4, 0:KD], tp4)

            pair_ps = {}
            pair_sb = {}

            def score_block(kb):
                # sT_kb[c, r]: keys of block kb vs queries of blocks kb,kb+1
                j, half = divmod(kb, 2)
                if half == 0:
                    pair_ps[j] = ps_pair.tile([P, 4 * P], f32, tag="s", name="s_ps")
                nc.tensor.matmul(
                    pair_ps[j][:, half * 2 * P : (half + 1) * 2 * P],
                    kT_sb[:, ts(kb, P)],
                    qT_sb[:, kb * P : kb * P + 2 * P],
                    start=True, stop=True,
                )
                if kb >= NB - 2:
                    # tail blocks: per-block exp+mask so the last chain is
                    # as short as possible
                    p1 = work.tile([P, 2 * P], bf, tag="p_sb")
                    nc.scalar.activation(
                        out=p1, in_=pair_ps[j][:, half * 2 * P : (half + 1) * 2 * P],
                        func=mybir.ActivationFunctionType.Exp,
                    )
                    nc.vector.tensor_mul(p1, p1, msk_sb[:, 0 : 2 * P])
                    pair_sb[kb + 100] = p1
                    if half == 1:
                        pair_ps.pop(j)
                elif half == 1:
                    p_sb = work.tile([P, 4 * P], bf, tag="p_sb")
                    nc.scalar.activation(
                        out=p_sb, in_=pair_ps.pop(j),
                        func=mybir.ActivationFunctionType.Exp,
                    )
                    # band+causal: per 256-block, cols 0:128 keep c <= r
                    # (diag qb=kb), cols 128:256 keep c >= r (qb=kb+1)
                    nc.vector.tensor_mul(p_sb, p_sb, msk_sb)
                    pair_sb[j] = p_sb

            def p_half(kb, h):
                # h=0: diag block of qb=kb; h=1: off-diag block of qb=kb+1
                if kb + 100 in pair_sb:
                    return pair_sb[kb + 100][:, h * P : (h + 1) * P]
                base = (kb % 2) * 2 * P + h * P
                return pair_sb[kb // 2][:, base : base + P]

            o4 = [None]

            def attend(qb):
                if qb % 4 == 0:
                    o4[0] = ps_small.tile([P, 4, OC], f32, tag="small", name="o4")
                op = o4[0][:, qb % 4]
                halves = [(qb, 0)]
                if qb > 0:
                    halves.insert(0, (qb - 1, 1))
                for i, (kb2, h) in enumerate(halves):
                    nc.tensor.matmul(
                        op,
                        p_half(kb2, h),
                        v_aug[:, kb2, :],
                        start=(i == 0),
                        stop=(i == len(halves) - 1),
                    )
                if qb >= 3:
                    pair_sb.pop((qb - 3) // 2, None)
                # stage+emit: groups of 4 for blocks 0-11, then 2+2 to
                # shorten the final-DMA tail
                emit = {3: (0, 4), 7: (4, 4), 11: (8, 4), 13: (12, 2), 15: (14, 2)}
                if qb in emit:
                    lo, cnt = emit[qb]
                    gsl = slice(lo, lo + cnt)
                    nc.vector.tensor_copy(oaug[:, gsl, :], o4[0][:, lo % 4 : lo % 4 + cnt])
                    nc.sync.dma_start(out=out[:, gsl, :], in_=oaug[:, gsl, :])

            # ---- software-pipelined schedule over the 4 column pieces
            scored = 0
            attended = 0
            for n in range(NCH):
                proj_piece(n)
                transpose_piece(n)
                target = min(4 * n + 2, NB - 1) if n < NCH - 1 else NB - 1
                while scored <= target:
                    score_block(scored)
                    scored += 1
                    # interleave attends while waiting on later pieces; in
                    # the last piece issue all scores first (scores never
                    # stall; attends can wait on the exp/mask chain)
                    if n < NCH - 1 and scored - attended > 3:
                        attend(attended)
                        attended += 1
            while attended < NB:
                attend(attended)
                attended += 1

    nc.finalize()
    return nc


def _prep_core_inputs(inputs):
    bfn = ml_dtypes.bfloat16
    g = lambda k: np.asarray(inputs[k], dtype=np.float32)
    x = g("x")
    scale = 1.0 / np.sqrt(np.float32(KD))
    temp = float(np.asarray(inputs["temperature"]).reshape(-1)[0])
    alpha = scale * temp  # folded (softmax temp) * (score scale)

    wq = np.concatenate([g("Wqr"), g("Wqi")], axis=1) * (scale * alpha)
    wk = np.concatenate([g("Wkr"), -g("Wki")], axis=1)
    wall = np.concatenate([wq, wk, g("Wv")], axis=1)  # [D, 320]
    wall = np.ascontiguousarray(
        wall.reshape(DCH, P, WCOL).transpose(1, 0, 2).astype(bfn)
    )

    pq = np.concatenate(
        [
            g("pos_qr") * alpha + g("bqr") * (scale * alpha),
            g("pos_qi") * alpha + g("bqi") * (scale * alpha),
        ],
        axis=1,
    ).T  # [128, S]
    pk = np.concatenate(
        [g("pos_kr") + g("bkr"), -(g("pos_ki") + g("bki"))], axis=1
    ).T
    ppack = np.stack([pq, pk], axis=0).astype(bfn)  # [2, P, S]

    cc, rr = np.meshgrid(np.arange(P), np.arange(P), indexing="ij")
    cpack = np.zeros((P, CCOL), dtype=np.float32)
    cpack[0:KD, 0:KD] = np.eye(KD)
    for rep in range(2):
        base = KD + rep * 2 * P
        cpack[:, base : base + P] = (cc <= rr)
        cpack[:, base + P : base + 2 * P] = (cc >= rr)
    cpack = cpack.astype(bfn)

    shared = {
        "wall": wall,
        "ppack": np.ascontiguousarray(ppack),
        "cpack": np.ascontiguousarray(cpack),
    }
    in_maps = []
    for b in range(NCORES):
        m = dict(shared)
        # xtr[n, q, p, c, j] = x[b].T[(2q+c)*128+p, n*512+j]
        xT_b = x[b].T.astype(bfn)
        m["xtr"] = np.ascontiguousarray(
            xT_b.reshape(2, 2, P, NCH, NSL).transpose(3, 0, 2, 1, 4)
        )
        in_maps.append(m)
    return in_maps


def kernel(**inputs):
    from concourse.bass_utils import run_bass_kernel_spmd

    nc = _CACHE.get("nc")
    if nc is None:
        nc = _CACHE["nc"] = _build_nc()
    in_maps = _prep_core_inputs(inputs)
    res = run_bass_kernel_spmd(
        nc, in_maps, core_ids=list(range(NCORES)), **TRACE_KWARGS
    )
    _CACHE["last_result"] = res
    bv = np.asarray(inputs["bv"], dtype=np.float32)
    outs = []
    for b in range(NCORES):
        arr = np.asarray(res.results[b]["out"]).astype(np.float32)  # [P,NB,OC]
        o = arr[:, :, 0:KD] / arr[:, :, KD : KD + 1] + bv
        outs.append(o.transpose(1, 0, 2).reshape(S, KD))
    return np.stack(outs, axis=0)
